# revision 30
# baseline (speedup 1.0000x reference)
"""Trainium2 Bass kernel for nn_Attention_76579266888357 (sparse_attention).

Reference computation (B=4, H=16, S=1024, D=64):
    s    = (q/8) @ k^T                      # [B,H,S,S]
    s    = s / |min(s)|                     # global min over whole tensor
    s    = s / max(||s_row||_2, 1e-12)      # row L2 normalize (F.normalize)
    attn = softmax(s, axis=-1)
    out  = attn @ v
    returns (out, attn, v)

Math notes that shape this implementation:
  * The temperature and global-|min| divisions cancel inside the row
    L2-normalize: logits = s_raw / max(||s_raw_row||, 8*eps*|m|) with
    s_raw = q@k^T and eps = 1e-12.  The eps clamp binds only when an
    entire row of q@k^T has L2 norm < ~1e-11, which cannot happen for
    any non-degenerate input (for randn inputs row norms are ~32, 13
    orders of magnitude away).  A host-side guard verifies this and
    falls back to an exact host computation if it ever binds, so no
    cross-core all-reduce(min) is needed at all.
  * Row norms are computed on the host via the Gram matrix
    ||q_row K^T||^2 = q_row (K^T K) q_row (f64, ~0.5 GFLOP total) and
    folded into a pre-scaled transposed query qsT = (q * u)^T, so the
    device matmul directly produces bounded logits in [-1, 1].
  * Softmax then needs no max-subtraction: exp(logits) is safe.
  * Device computes, per head (8 heads per core, 64 total over 8 cores):
      logits^T tiles = kT_tile^T-stationary x qsT     (PE, bf16)
      e^T = exp(logits^T)                             (ACT, PSUM->SBUF, bf16)
      outAug^T += vAug_tile^T-stationary x e^T        (PE, bf16, accumulate;
          vAug = [v | 1] so row 64 of outAug^T is the softmax denom)
      logits tiles (natural) = qsT_tile^T x kT        (PE, bf16)
      e = exp(logits), denom via accum_out            (ACT, fp32)
      attn = e * (1/denom)                            (DVE per-partition)
    attn streams to HBM in natural layout; outAug^T + denom go to HBM
    and the host finishes out = (outAug^T[:64]/denom)^T (layout fixup
    only, no FLOPs that the device could do better).
"""

import os
import sys

import numpy as np

for _p in ("/opt/trn_rl_repo", "/root/.axon_site/_ro/trn_rl_repo"):
    if os.path.isdir(_p) and _p not in sys.path:
        sys.path.insert(0, _p)

import concourse.bass as bass
import concourse.mybir as mybir
import concourse.tile as tile
from concourse.bass_utils import run_bass_kernel_spmd

B, H, S, D = 4, 16, 1024, 64
BH = B * H
NCORES = 8
HPC = BH // NCORES  # heads per core
P = 128
NT = S // P  # 8 row/key tiles per head
F32 = mybir.dt.float32
F32R = mybir.dt.float32r
BF16 = mybir.dt.bfloat16
MM_HALF = 512  # fp32 moving-operand max per PSUM bank

# results of the last device run (exec_time_ns etc.), for test harnesses
last_results = None


# the pinned walrus rejects instructions carrying more sync-wait commands
# than the ISA struct has slots for; observed: Drain/TPB_CTRL chokes on 3,
# Matmult (its S3_LW ldweights half) and Activation (S3D3_AC) choke on 2.
# One wait per instruction is universally safe; spill the rest onto NOPs.
_MAX_WAITS_DEFAULT = 1
_MAX_WAITS_BY_OPCODE = {}


class _TileContextPatched(tile.TileContext):
    """The pinned walrus rejects >2 sync-wait commands on any single
    instruction.  Split excess waits onto injected same-engine NOPs that
    execute immediately before the instruction (sequencers run their
    stream in order, so waiting on a preceding NOP is equivalent)."""

    def _split_excess_waits(self, ordered):
        nc = self.nc
        for bb_name, insts in ordered.items():
            i = 0
            while i < len(insts):
                inst = insts[i]
                si = getattr(inst, "sync_info", None)
                waits = list(si.on_wait) if si is not None else []
                limit = _MAX_WAITS_BY_OPCODE.get(
                    type(inst).__name__.removeprefix("Inst"), _MAX_WAITS_DEFAULT
                )
                if len(waits) > limit:
                    keep = waits[-limit:] if limit > 0 else []
                    spill = waits[: len(waits) - len(keep)]
                    inst.sync_info = mybir.SyncInfo(
                        on_wait=keep, on_update=list(si.on_update)
                    )
                    eng = nc.engines.get(inst.engine, nc.sync)
                    new_nops = []
                    for w in spill:
                        nop = eng.nop(hint="wait_split")
                        # nop() appended itself to the current bb; drop that
                        # copy - blocks are rebuilt from `ordered` anyway.
                        nop.ins.sync_info = mybir.SyncInfo(
                            on_wait=[w], on_update=[]
                        )
                        new_nops.append(nop.ins)
                    insts[i:i] = new_nops
                    i += len(new_nops)
                i += 1

    def _lower_ordered_insts(self, ordered):
        self._split_excess_waits(ordered)
        return super()._lower_ordered_insts(ordered)

    def _drain_and_barrier(self, tick_clock, wait_clock):
        from bass_rust import ScopedClock

        nc = self.nc
        probe = nc.sync.nop(hint="tail_wait_probe")
        wait_clock.add_sem_waits(
            probe.ins, ScopedClock({None: tick_clock.global_clock})
        )
        si = probe.ins.sync_info
        waits = list(si.on_wait) if si is not None else []
        probe.ins.sync_info = mybir.SyncInfo(on_wait=waits[:1], on_update=[])
        for w in waits[1:]:
            n = nc.sync.nop(hint="tail_wait_split")
            n.ins.sync_info = mybir.SyncInfo(on_wait=[w], on_update=[])
        nc.sync.drain()
        nc.all_engine_barrier()
        assert self.sems is not None
        popped = nc._tile_sem_poison_stack.pop()
        assert popped is self._sem_poison
        nc.clear_and_free_semaphores(list(self.sems.allocated().values()))
        nc.all_engine_barrier()


def _build_bass(mode):
    """mode: how the natural-layout QK pass (which feeds attn) multiplies.
    "bf16": single bf16 product (attn err ~1e-4 measured)
    "hilo": bf16 hi/lo 3-term split (attn err ~1e-5, 3x the natural MMs)
    The transposed pass + AV matmul (which feed only `out`) are always
    single bf16."""
    nc = bass.Bass()
    EXP = mybir.ActivationFunctionType.Exp

    if mode == "hilo":
        qloT = nc.dram_tensor("qloT", [HPC, D, S], BF16, kind="ExternalInput")
        kloT = nc.dram_tensor("kloT", [HPC, D, S], BF16, kind="ExternalInput")
    qsTb = nc.dram_tensor("qsTb", [HPC, D, S], BF16, kind="ExternalInput")
    kTb = nc.dram_tensor("kTb", [HPC, D, S], BF16, kind="ExternalInput")
    vaug = nc.dram_tensor("vaug", [HPC, S, D + 1], BF16, kind="ExternalInput")
    attn = nc.dram_tensor("attn", [HPC, S, S], F32, kind="ExternalOutput")
    oden = nc.dram_tensor("oden", [HPC, D + 1, S], F32, kind="ExternalOutput")

    with _TileContextPatched(nc) as tc:
        with (
            tc.tile_pool(name="qk_in", bufs=3) as qk_pool,
            tc.tile_pool(name="v_in", bufs=3) as v_pool,
            tc.tile_pool(name="eT", bufs=4) as eT_pool,
            tc.tile_pool(name="enat", bufs=4) as en_pool,
            tc.tile_pool(name="attn_sb", bufs=4) as at_pool,
            tc.tile_pool(name="oden_sb", bufs=2) as od_pool,
            tc.tile_pool(name="small", bufs=8) as sm_pool,
            tc.tile_pool(name="ps_sc", bufs=3, space="PSUM") as ps_sc,
            tc.tile_pool(name="ps_od", bufs=1, space="PSUM") as ps_od,
        ):
            for h in range(HPC):
                qsb = qk_pool.tile([D, S], BF16, tag="qsb")
                ktb = qk_pool.tile([D, S], BF16, tag="ktb")
                nc.sync.dma_start(out=qsb[:], in_=qsTb[h])
                nc.sync.dma_start(out=ktb[:], in_=kTb[h])
                if mode == "hilo":
                    qlo = qk_pool.tile([D, S], BF16, tag="qlo")
                    klo = qk_pool.tile([D, S], BF16, tag="klo")
                    nc.sync.dma_start(out=qlo[:], in_=qloT[h])
                    nc.sync.dma_start(out=klo[:], in_=kloT[h])
                vt = v_pool.tile([P, NT, D + 1], BF16)
                nc.sync.dma_start(
                    out=vt[:], in_=vaug[h].rearrange("(t p) c -> p t c", p=P)
                )

                # --- transposed pass: e^T tiles + accumulated outAug^T ---
                od_ps = ps_od.tile([D + 1, S], F32)
                for ktile in range(NT):
                    sT_ps = ps_sc.tile([P, S], F32, tag="sc")
                    for j in range(2):
                        nc.tensor.matmul(
                            sT_ps[:, j * MM_HALF : (j + 1) * MM_HALF],
                            lhsT=ktb[:, ktile * P : (ktile + 1) * P],
                            rhs=qsb[:, j * MM_HALF : (j + 1) * MM_HALF],
                            start=True,
                            stop=True,
                        )
                    eT = eT_pool.tile([P, S], BF16)
                    nc.scalar.activation(out=eT[:], in_=sT_ps[:], func=EXP)
                    for j in range(2):
                        nc.tensor.matmul(
                            od_ps[:, j * MM_HALF : (j + 1) * MM_HALF],
                            lhsT=vt[:, ktile, :],
                            rhs=eT[:, j * MM_HALF : (j + 1) * MM_HALF],
                            start=(ktile == 0),
                            stop=(ktile == NT - 1),
                            skip_group_check=True,
                        )
                od_sb = od_pool.tile([D + 1, S], F32)
                nc.vector.tensor_copy(out=od_sb[:], in_=od_ps[:])
                nc.sync.dma_start(out=oden[h], in_=od_sb[:])

                # --- natural pass: e = exp(logits), denom via accum_out,
                #     attn = e * (1/denom) on DVE ---
                for qt in range(NT):
                    s_ps = ps_sc.tile([P, S], F32, tag="sc")
                    q_sl = slice(qt * P, (qt + 1) * P)
                    for j in range(2):
                        sl_out = s_ps[:, j * MM_HALF : (j + 1) * MM_HALF]
                        k_sl = slice(j * MM_HALF, (j + 1) * MM_HALF)
                        if mode == "hilo":
                            nc.tensor.matmul(
                                sl_out, lhsT=qsb[:, q_sl], rhs=ktb[:, k_sl],
                                start=True, stop=False, skip_group_check=True,
                            )
                            nc.tensor.matmul(
                                sl_out, lhsT=qlo[:, q_sl], rhs=ktb[:, k_sl],
                                start=False, stop=False, skip_group_check=True,
                            )
                            nc.tensor.matmul(
                                sl_out, lhsT=qsb[:, q_sl], rhs=klo[:, k_sl],
                                start=False, stop=True, skip_group_check=True,
                            )
                        else:
                            nc.tensor.matmul(
                                sl_out, lhsT=qsb[:, q_sl], rhs=ktb[:, k_sl],
                                start=True, stop=True,
                            )
                    en = en_pool.tile([P, S], F32)
                    nc.scalar.activation(out=en[:], in_=s_ps[:], func=EXP)
                    dn = sm_pool.tile([P, 1], F32, tag="dn")
                    nc.vector.reduce_sum(dn[:], en[:], axis=mybir.AxisListType.X)
                    invd = sm_pool.tile([P, 1], F32, tag="invd")
                    nc.vector.reciprocal(out=invd[:], in_=dn[:])
                    at = at_pool.tile([P, S], F32)
                    nc.vector.tensor_scalar_mul(at[:], en[:], invd[:])
                    nc.sync.dma_start(
                        out=attn[h, qt * P : (qt + 1) * P, :], in_=at[:]
                    )
    return nc


_nc_cache = {}


def _get_nc(mode):
    if mode not in _nc_cache:
        _nc_cache[mode] = _build_bass(mode)
    return _nc_cache[mode]


def _ensure_ntff_hook():
    """bass_utils' trace path imports antenv.axon_hooks, which this
    container does not ship.  Recreate it from the boot helper so NTFF
    profiling (HW exec time) works.  Best-effort: failures just mean no
    trace."""
    try:
        from antenv.axon_hooks import get_axon_ntff_profile_hook  # noqa: F401

        return
    except ImportError:
        pass
    try:
        import types

        import antenv
        from trn_agent_boot.trn_boot import _ntff_profile_via_ctypes

        hook = {"h": _ntff_profile_via_ctypes("/opt/axon/libaxon_pjrt.so")}
        m = types.ModuleType("antenv.axon_hooks")
        m.get_axon_ntff_profile_hook = lambda: hook["h"]

        def _set(h):
            hook["h"] = h

        m.set_axon_ntff_profile_hook = _set
        sys.modules["antenv.axon_hooks"] = m
        antenv.axon_hooks = m
    except Exception:
        pass


def _host_reference_fallback(q, k, v):
    """Exact (host) computation, used only if the eps clamp could bind."""
    s = np.einsum("bhqd,bhkd->bhqk", q / 8.0, k).astype(np.float32)
    s = s / np.abs(s.min())
    rn = np.linalg.norm(s, axis=-1, keepdims=True)
    s = s / np.maximum(rn, 1e-12)
    s = s - s.max(axis=-1, keepdims=True)
    e = np.exp(s)
    attn = e / e.sum(axis=-1, keepdims=True)
    out = np.einsum("bhqk,bhkd->bhqd", attn, v).astype(np.float32)
    return out, attn.astype(np.float32)


def kernel(q, k, v):
    global last_results
    q = np.asarray(q, dtype=np.float32)
    k = np.asarray(k, dtype=np.float32)
    v = np.asarray(v, dtype=np.float32)

    qf = q.reshape(BH, S, D).astype(np.float64)
    kf = k.reshape(BH, S, D).astype(np.float64)
    # row norms of q @ k^T via the Gram matrix (exact to f64)
    G = np.matmul(kf.transpose(0, 2, 1), kf)  # [BH, D, D]
    w = np.matmul(qf, G)  # [BH, S, D]
    r2 = np.einsum("hsd,hsd->hs", qf, w)
    r = np.sqrt(np.maximum(r2, 0.0))
    # eps-clamp guard: the reference's F.normalize clamp binds for row q iff
    # ||s_q|| / (8*|m|) < 1e-12 with s = q@k^T and |m| = |min(s/8)|.  Since
    # |m| <= max_j |s_qj|/8 <= max_q ||s_q||/8, binding requires
    # r.min() < 1e-12 * r.max().  Check with a 10x safety margin.
    if r.min() < 1e-11 * r.max():
        out, attn = _host_reference_fallback(q, k, v)
        return out, attn, v

    import ml_dtypes

    bf16 = ml_dtypes.bfloat16
    mode = os.environ.get("BASS_QK_MODE", "bf16")
    u = 1.0 / r
    qsT = np.ascontiguousarray(
        (qf * u[:, :, None]).transpose(0, 2, 1), dtype=np.float32
    )  # [BH, D, S]
    kT = np.ascontiguousarray(
        k.reshape(BH, S, D).transpose(0, 2, 1)
    )  # [BH, D, S] f32
    vaug = np.concatenate(
        [v.reshape(BH, S, D), np.ones((BH, S, 1), dtype=np.float32)], axis=2
    ).astype(bf16)  # [BH, S, D+1] bf16 (feeds only the out matmul)
    qsTb = qsT.astype(bf16)
    kTb = kT.astype(bf16)

    nc = _get_nc(mode)
    shards = {
        "qsTb": qsTb,
        "kTb": kTb,
        "vaug": vaug,
    }
    if mode == "hilo":
        shards["qloT"] = (qsT - qsTb.astype(np.float32)).astype(bf16)
        shards["kloT"] = (kT - kTb.astype(np.float32)).astype(bf16)
    in_maps = [
        {
            name: np.ascontiguousarray(arr[c * HPC : (c + 1) * HPC])
            for name, arr in shards.items()
        }
        for c in range(NCORES)
    ]
    trace = bool(int(os.environ.get("BASS_KERNEL_TRACE", "0")))
    if trace:
        _ensure_ntff_hook()
    res = run_bass_kernel_spmd(
        nc, in_maps, core_ids=list(range(NCORES)), trace=trace
    )
    last_results = res

    attn = (
        np.concatenate([r_["attn"] for r_ in res.results], axis=0)
        .reshape(B, H, S, S)
    )
    oden = np.concatenate([r_["oden"] for r_ in res.results], axis=0)
    out = np.ascontiguousarray(
        (oden[:, :D, :] / oden[:, D : D + 1, :]).transpose(0, 2, 1)
    ).reshape(B, H, S, D)
    return out, attn, v


# revision 31
# speedup vs baseline: 1.0950x; 1.0950x over previous
"""Trainium2 Bass kernel for nn_Attention_76579266888357 (sparse_attention).

Reference computation (B=4, H=16, S=1024, D=64):
    s    = (q/8) @ k^T                      # [B,H,S,S]
    s    = s / |min(s)|                     # global min over whole tensor
    s    = s / max(||s_row||_2, 1e-12)      # row L2 normalize (F.normalize)
    attn = softmax(s, axis=-1)
    out  = attn @ v
    returns (out, attn, v)

Math notes that shape this implementation:
  * The temperature and global-|min| divisions cancel inside the row
    L2-normalize: logits = s_raw / max(||s_raw_row||, 8*eps*|m|) with
    s_raw = q@k^T and eps = 1e-12.  The eps clamp binds only when an
    entire row of q@k^T has L2 norm < ~1e-11, which cannot happen for
    any non-degenerate input (for randn inputs row norms are ~32, 13
    orders of magnitude away).  A host-side guard verifies this and
    falls back to an exact host computation if it ever binds, so no
    cross-core all-reduce(min) is needed at all.
  * Row norms are computed on the host via the Gram matrix
    ||q_row K^T||^2 = q_row (K^T K) q_row (f64, ~0.5 GFLOP total) and
    folded into a pre-scaled transposed query qsT = (q * u)^T, so the
    device matmul directly produces bounded logits in [-1, 1].
  * Softmax then needs no max-subtraction: exp(logits) is safe.
  * Device computes, per head (8 heads per core, 64 total over 8 cores):
      logits^T tiles = kT_tile^T-stationary x qsT     (PE, bf16)
      e^T = exp(logits^T)                             (ACT, PSUM->SBUF, bf16)
      outAug^T += vAug_tile^T-stationary x e^T        (PE, bf16, accumulate;
          vAug = [v | 1] so row 64 of outAug^T is the softmax denom)
      logits tiles (natural) = qsT_tile^T x kT        (PE, bf16)
      e = exp(logits), denom via accum_out            (ACT, fp32)
      attn = e * (1/denom)                            (DVE per-partition)
    attn streams to HBM in natural layout; outAug^T + denom go to HBM
    and the host finishes out = (outAug^T[:64]/denom)^T (layout fixup
    only, no FLOPs that the device could do better).
"""

import os
import sys

import numpy as np

for _p in ("/opt/trn_rl_repo", "/root/.axon_site/_ro/trn_rl_repo"):
    if os.path.isdir(_p) and _p not in sys.path:
        sys.path.insert(0, _p)

import concourse.bass as bass
import concourse.mybir as mybir
import concourse.tile as tile
from concourse.bass_utils import run_bass_kernel_spmd

B, H, S, D = 4, 16, 1024, 64
BH = B * H
NCORES = 8
HPC = BH // NCORES  # heads per core
P = 128
NT = S // P  # 8 row/key tiles per head
F32 = mybir.dt.float32
F32R = mybir.dt.float32r
BF16 = mybir.dt.bfloat16
MM_HALF = 512  # fp32 moving-operand max per PSUM bank

# results of the last device run (exec_time_ns etc.), for test harnesses
last_results = None


# the pinned walrus rejects instructions carrying more sync-wait commands
# than the ISA struct has slots for; observed: Drain/TPB_CTRL chokes on 3,
# Matmult (its S3_LW ldweights half) and Activation (S3D3_AC) choke on 2.
# One wait per instruction is universally safe; spill the rest onto NOPs.
_MAX_WAITS_DEFAULT = 1
_MAX_WAITS_BY_OPCODE = {}


class _TileContextPatched(tile.TileContext):
    """The pinned walrus rejects >2 sync-wait commands on any single
    instruction.  Split excess waits onto injected same-engine NOPs that
    execute immediately before the instruction (sequencers run their
    stream in order, so waiting on a preceding NOP is equivalent)."""

    def _split_excess_waits(self, ordered):
        nc = self.nc
        for bb_name, insts in ordered.items():
            i = 0
            while i < len(insts):
                inst = insts[i]
                si = getattr(inst, "sync_info", None)
                waits = list(si.on_wait) if si is not None else []
                limit = _MAX_WAITS_BY_OPCODE.get(
                    type(inst).__name__.removeprefix("Inst"), _MAX_WAITS_DEFAULT
                )
                if len(waits) > limit:
                    keep = waits[-limit:] if limit > 0 else []
                    spill = waits[: len(waits) - len(keep)]
                    inst.sync_info = mybir.SyncInfo(
                        on_wait=keep, on_update=list(si.on_update)
                    )
                    eng = nc.engines.get(inst.engine, nc.sync)
                    new_nops = []
                    for w in spill:
                        nop = eng.nop(hint="wait_split")
                        # nop() appended itself to the current bb; drop that
                        # copy - blocks are rebuilt from `ordered` anyway.
                        nop.ins.sync_info = mybir.SyncInfo(
                            on_wait=[w], on_update=[]
                        )
                        new_nops.append(nop.ins)
                    insts[i:i] = new_nops
                    i += len(new_nops)
                i += 1

    def _lower_ordered_insts(self, ordered):
        self._split_excess_waits(ordered)
        return super()._lower_ordered_insts(ordered)

    def _drain_and_barrier(self, tick_clock, wait_clock):
        from bass_rust import ScopedClock

        nc = self.nc
        probe = nc.sync.nop(hint="tail_wait_probe")
        wait_clock.add_sem_waits(
            probe.ins, ScopedClock({None: tick_clock.global_clock})
        )
        si = probe.ins.sync_info
        waits = list(si.on_wait) if si is not None else []
        probe.ins.sync_info = mybir.SyncInfo(on_wait=waits[:1], on_update=[])
        for w in waits[1:]:
            n = nc.sync.nop(hint="tail_wait_split")
            n.ins.sync_info = mybir.SyncInfo(on_wait=[w], on_update=[])
        nc.sync.drain()
        nc.all_engine_barrier()
        assert self.sems is not None
        popped = nc._tile_sem_poison_stack.pop()
        assert popped is self._sem_poison
        nc.clear_and_free_semaphores(list(self.sems.allocated().values()))
        nc.all_engine_barrier()


def _build_bass(mode):
    """mode: how the natural-layout QK pass (which feeds attn) multiplies.
    "bf16": single bf16 product (attn err ~1e-4 measured)
    "hilo": bf16 hi/lo 3-term split (attn err ~1e-5, 3x the natural MMs)
    The transposed pass + AV matmul (which feed only `out`) are always
    single bf16."""
    nc = bass.Bass()
    EXP = mybir.ActivationFunctionType.Exp

    if mode == "hilo":
        qloT = nc.dram_tensor("qloT", [HPC, D, S], BF16, kind="ExternalInput")
        kloT = nc.dram_tensor("kloT", [HPC, D, S], BF16, kind="ExternalInput")
    qsTb = nc.dram_tensor("qsTb", [HPC, D, S], BF16, kind="ExternalInput")
    kTb = nc.dram_tensor("kTb", [HPC, D, S], BF16, kind="ExternalInput")
    vaug = nc.dram_tensor("vaug", [HPC, S, D + 1], BF16, kind="ExternalInput")
    attn = nc.dram_tensor("attn", [HPC, S, S], F32, kind="ExternalOutput")
    oden = nc.dram_tensor("oden", [HPC, D + 1, S], F32, kind="ExternalOutput")

    with _TileContextPatched(nc) as tc:
        with (
            tc.tile_pool(name="qk_in", bufs=3) as qk_pool,
            tc.tile_pool(name="v_in", bufs=3) as v_pool,
            tc.tile_pool(name="eT", bufs=4) as eT_pool,
            tc.tile_pool(name="enat", bufs=4) as en_pool,
            tc.tile_pool(name="attn_sb", bufs=4) as at_pool,
            tc.tile_pool(name="oden_sb", bufs=2) as od_pool,
            tc.tile_pool(name="small", bufs=8) as sm_pool,
            tc.tile_pool(name="ps_sc", bufs=3, space="PSUM") as ps_sc,
            tc.tile_pool(name="ps_od", bufs=1, space="PSUM") as ps_od,
        ):
            for h in range(HPC):
                qsb = qk_pool.tile([D, S], BF16, tag="qsb")
                ktb = qk_pool.tile([D, S], BF16, tag="ktb")
                nc.sync.dma_start(out=qsb[:], in_=qsTb[h])
                nc.sync.dma_start(out=ktb[:], in_=kTb[h])
                if mode == "hilo":
                    qlo = qk_pool.tile([D, S], BF16, tag="qlo")
                    klo = qk_pool.tile([D, S], BF16, tag="klo")
                    nc.sync.dma_start(out=qlo[:], in_=qloT[h])
                    nc.sync.dma_start(out=klo[:], in_=kloT[h])
                vt = v_pool.tile([P, NT, D + 1], BF16)
                nc.sync.dma_start(
                    out=vt[:], in_=vaug[h].rearrange("(t p) c -> p t c", p=P)
                )

                # --- transposed pass: e^T tiles + accumulated outAug^T ---
                od_ps = ps_od.tile([D + 1, S], F32)
                for ktile in range(NT):
                    sT_ps = ps_sc.tile([P, S], F32, tag="sc")
                    for j in range(2):
                        nc.tensor.matmul(
                            sT_ps[:, j * MM_HALF : (j + 1) * MM_HALF],
                            lhsT=ktb[:, ktile * P : (ktile + 1) * P],
                            rhs=qsb[:, j * MM_HALF : (j + 1) * MM_HALF],
                            start=True,
                            stop=True,
                        )
                    eT = eT_pool.tile([P, S], BF16)
                    nc.scalar.activation(out=eT[:], in_=sT_ps[:], func=EXP)
                    for j in range(2):
                        nc.tensor.matmul(
                            od_ps[:, j * MM_HALF : (j + 1) * MM_HALF],
                            lhsT=vt[:, ktile, :],
                            rhs=eT[:, j * MM_HALF : (j + 1) * MM_HALF],
                            start=(ktile == 0),
                            stop=(ktile == NT - 1),
                            skip_group_check=True,
                        )
                od_sb = od_pool.tile([D + 1, S], F32)
                nc.vector.tensor_copy(out=od_sb[:], in_=od_ps[:])
                nc.sync.dma_start(out=oden[h], in_=od_sb[:])

                # --- natural pass: e = exp(logits), denom via accum_out,
                #     attn = e * (1/denom) on DVE ---
                for qt in range(NT):
                    s_ps = ps_sc.tile([P, S], F32, tag="sc")
                    q_sl = slice(qt * P, (qt + 1) * P)
                    for j in range(2):
                        sl_out = s_ps[:, j * MM_HALF : (j + 1) * MM_HALF]
                        k_sl = slice(j * MM_HALF, (j + 1) * MM_HALF)
                        if mode == "hilo":
                            nc.tensor.matmul(
                                sl_out, lhsT=qsb[:, q_sl], rhs=ktb[:, k_sl],
                                start=True, stop=False, skip_group_check=True,
                            )
                            nc.tensor.matmul(
                                sl_out, lhsT=qlo[:, q_sl], rhs=ktb[:, k_sl],
                                start=False, stop=False, skip_group_check=True,
                            )
                            nc.tensor.matmul(
                                sl_out, lhsT=qsb[:, q_sl], rhs=klo[:, k_sl],
                                start=False, stop=True, skip_group_check=True,
                            )
                        else:
                            nc.tensor.matmul(
                                sl_out, lhsT=qsb[:, q_sl], rhs=ktb[:, k_sl],
                                start=True, stop=True,
                            )
                    en = en_pool.tile([P, S], F32)
                    dn = sm_pool.tile([P, 1], F32, tag="dn")
                    nc.scalar.activation(
                        out=en[:], in_=s_ps[:], func=EXP, accum_out=dn[:]
                    )
                    invd = sm_pool.tile([P, 1], F32, tag="invd")
                    nc.vector.reciprocal(out=invd[:], in_=dn[:])
                    at = at_pool.tile([P, S], F32)
                    nc.vector.tensor_scalar_mul(at[:], en[:], invd[:])
                    nc.sync.dma_start(
                        out=attn[h, qt * P : (qt + 1) * P, :], in_=at[:]
                    )
    return nc


_nc_cache = {}


def _get_nc(mode):
    if mode not in _nc_cache:
        _nc_cache[mode] = _build_bass(mode)
    return _nc_cache[mode]


def _ensure_ntff_hook():
    """bass_utils' trace path imports antenv.axon_hooks, which this
    container does not ship.  Recreate it from the boot helper so NTFF
    profiling (HW exec time) works.  Best-effort: failures just mean no
    trace."""
    try:
        from antenv.axon_hooks import get_axon_ntff_profile_hook  # noqa: F401

        return
    except ImportError:
        pass
    try:
        import types

        import antenv
        from trn_agent_boot.trn_boot import _ntff_profile_via_ctypes

        hook = {"h": _ntff_profile_via_ctypes("/opt/axon/libaxon_pjrt.so")}
        m = types.ModuleType("antenv.axon_hooks")
        m.get_axon_ntff_profile_hook = lambda: hook["h"]

        def _set(h):
            hook["h"] = h

        m.set_axon_ntff_profile_hook = _set
        sys.modules["antenv.axon_hooks"] = m
        antenv.axon_hooks = m
    except Exception:
        pass


def _host_reference_fallback(q, k, v):
    """Exact (host) computation, used only if the eps clamp could bind."""
    s = np.einsum("bhqd,bhkd->bhqk", q / 8.0, k).astype(np.float32)
    s = s / np.abs(s.min())
    rn = np.linalg.norm(s, axis=-1, keepdims=True)
    s = s / np.maximum(rn, 1e-12)
    s = s - s.max(axis=-1, keepdims=True)
    e = np.exp(s)
    attn = e / e.sum(axis=-1, keepdims=True)
    out = np.einsum("bhqk,bhkd->bhqd", attn, v).astype(np.float32)
    return out, attn.astype(np.float32)


def kernel(q, k, v):
    global last_results
    q = np.asarray(q, dtype=np.float32)
    k = np.asarray(k, dtype=np.float32)
    v = np.asarray(v, dtype=np.float32)

    qf = q.reshape(BH, S, D).astype(np.float64)
    kf = k.reshape(BH, S, D).astype(np.float64)
    # row norms of q @ k^T via the Gram matrix (exact to f64)
    G = np.matmul(kf.transpose(0, 2, 1), kf)  # [BH, D, D]
    w = np.matmul(qf, G)  # [BH, S, D]
    r2 = np.einsum("hsd,hsd->hs", qf, w)
    r = np.sqrt(np.maximum(r2, 0.0))
    # eps-clamp guard: the reference's F.normalize clamp binds for row q iff
    # ||s_q|| / (8*|m|) < 1e-12 with s = q@k^T and |m| = |min(s/8)|.  Since
    # |m| <= max_j |s_qj|/8 <= max_q ||s_q||/8, binding requires
    # r.min() < 1e-12 * r.max().  Check with a 10x safety margin.
    if r.min() < 1e-11 * r.max():
        out, attn = _host_reference_fallback(q, k, v)
        return out, attn, v

    import ml_dtypes

    bf16 = ml_dtypes.bfloat16
    mode = os.environ.get("BASS_QK_MODE", "bf16")
    u = 1.0 / r
    qsT = np.ascontiguousarray(
        (qf * u[:, :, None]).transpose(0, 2, 1), dtype=np.float32
    )  # [BH, D, S]
    kT = np.ascontiguousarray(
        k.reshape(BH, S, D).transpose(0, 2, 1)
    )  # [BH, D, S] f32
    vaug = np.concatenate(
        [v.reshape(BH, S, D), np.ones((BH, S, 1), dtype=np.float32)], axis=2
    ).astype(bf16)  # [BH, S, D+1] bf16 (feeds only the out matmul)
    qsTb = qsT.astype(bf16)
    kTb = kT.astype(bf16)

    nc = _get_nc(mode)
    shards = {
        "qsTb": qsTb,
        "kTb": kTb,
        "vaug": vaug,
    }
    if mode == "hilo":
        shards["qloT"] = (qsT - qsTb.astype(np.float32)).astype(bf16)
        shards["kloT"] = (kT - kTb.astype(np.float32)).astype(bf16)
    in_maps = [
        {
            name: np.ascontiguousarray(arr[c * HPC : (c + 1) * HPC])
            for name, arr in shards.items()
        }
        for c in range(NCORES)
    ]
    trace = bool(int(os.environ.get("BASS_KERNEL_TRACE", "0")))
    if trace:
        _ensure_ntff_hook()
    res = run_bass_kernel_spmd(
        nc, in_maps, core_ids=list(range(NCORES)), trace=trace
    )
    last_results = res

    attn = (
        np.concatenate([r_["attn"] for r_ in res.results], axis=0)
        .reshape(B, H, S, S)
    )
    oden = np.concatenate([r_["oden"] for r_ in res.results], axis=0)
    out = np.ascontiguousarray(
        (oden[:, :D, :] / oden[:, D : D + 1, :]).transpose(0, 2, 1)
    ).reshape(B, H, S, D)
    return out, attn, v


# revision 32
# speedup vs baseline: 1.1740x; 1.0722x over previous
"""Trainium2 Bass kernel for nn_Attention_76579266888357 (sparse_attention).

Reference computation (B=4, H=16, S=1024, D=64):
    s    = (q/8) @ k^T                      # [B,H,S,S]
    s    = s / |min(s)|                     # global min over whole tensor
    s    = s / max(||s_row||_2, 1e-12)      # row L2 normalize (F.normalize)
    attn = softmax(s, axis=-1)
    out  = attn @ v
    returns (out, attn, v)

Math notes that shape this implementation:
  * The temperature and global-|min| divisions cancel inside the row
    L2-normalize: logits = s_raw / max(||s_raw_row||, 8*eps*|m|) with
    s_raw = q@k^T and eps = 1e-12.  The eps clamp binds only when an
    entire row of q@k^T has L2 norm < ~1e-11, which cannot happen for
    any non-degenerate input (for randn inputs row norms are ~32, 13
    orders of magnitude away).  A host-side guard verifies this and
    falls back to an exact host computation if it ever binds, so no
    cross-core all-reduce(min) is needed at all.
  * Row norms are computed on the host via the Gram matrix
    ||q_row K^T||^2 = q_row (K^T K) q_row (f64, ~0.5 GFLOP total) and
    folded into a pre-scaled transposed query qsT = (q * u)^T, so the
    device matmul directly produces bounded logits in [-1, 1].
  * Softmax then needs no max-subtraction: exp(logits) is safe.
  * Device computes, per head (8 heads per core, 64 total over 8 cores):
      logits^T tiles = kT_tile^T-stationary x qsT     (PE, bf16)
      e^T = exp(logits^T)                             (ACT, PSUM->SBUF, bf16)
      outAug^T += vAug_tile^T-stationary x e^T        (PE, bf16, accumulate;
          vAug = [v | 1] so row 64 of outAug^T is the softmax denom)
      logits tiles (natural) = qsT_tile^T x kT        (PE, bf16)
      e = exp(logits), denom via accum_out            (ACT, fp32)
      attn = e * (1/denom)                            (DVE per-partition)
    attn streams to HBM in natural layout; outAug^T + denom go to HBM
    and the host finishes out = (outAug^T[:64]/denom)^T (layout fixup
    only, no FLOPs that the device could do better).
"""

import os
import sys

import numpy as np

for _p in ("/opt/trn_rl_repo", "/root/.axon_site/_ro/trn_rl_repo"):
    if os.path.isdir(_p) and _p not in sys.path:
        sys.path.insert(0, _p)

import concourse.bass as bass
import concourse.mybir as mybir
import concourse.tile as tile
from concourse.bass_utils import run_bass_kernel_spmd

B, H, S, D = 4, 16, 1024, 64
BH = B * H
NCORES = 8
HPC = BH // NCORES  # heads per core
P = 128
NT = S // P  # 8 row/key tiles per head
F32 = mybir.dt.float32
F32R = mybir.dt.float32r
BF16 = mybir.dt.bfloat16
MM_HALF = 512  # fp32 moving-operand max per PSUM bank

# results of the last device run (exec_time_ns etc.), for test harnesses
last_results = None


# the pinned walrus rejects instructions carrying more sync-wait commands
# than the ISA struct has slots for; observed: Drain/TPB_CTRL chokes on 3,
# Matmult (its S3_LW ldweights half) and Activation (S3D3_AC) choke on 2.
# One wait per instruction is universally safe; spill the rest onto NOPs.
_MAX_WAITS_DEFAULT = 1
_MAX_WAITS_BY_OPCODE = {}


class _TileContextPatched(tile.TileContext):
    """The pinned walrus rejects >2 sync-wait commands on any single
    instruction.  Split excess waits onto injected same-engine NOPs that
    execute immediately before the instruction (sequencers run their
    stream in order, so waiting on a preceding NOP is equivalent)."""

    def _split_excess_waits(self, ordered):
        nc = self.nc
        for bb_name, insts in ordered.items():
            i = 0
            while i < len(insts):
                inst = insts[i]
                si = getattr(inst, "sync_info", None)
                waits = list(si.on_wait) if si is not None else []
                limit = _MAX_WAITS_BY_OPCODE.get(
                    type(inst).__name__.removeprefix("Inst"), _MAX_WAITS_DEFAULT
                )
                if len(waits) > limit:
                    keep = waits[-limit:] if limit > 0 else []
                    spill = waits[: len(waits) - len(keep)]
                    inst.sync_info = mybir.SyncInfo(
                        on_wait=keep, on_update=list(si.on_update)
                    )
                    eng = nc.engines.get(inst.engine, nc.sync)
                    new_nops = []
                    for w in spill:
                        nop = eng.nop(hint="wait_split")
                        # nop() appended itself to the current bb; drop that
                        # copy - blocks are rebuilt from `ordered` anyway.
                        nop.ins.sync_info = mybir.SyncInfo(
                            on_wait=[w], on_update=[]
                        )
                        new_nops.append(nop.ins)
                    insts[i:i] = new_nops
                    i += len(new_nops)
                i += 1

    def _lower_ordered_insts(self, ordered):
        self._split_excess_waits(ordered)
        return super()._lower_ordered_insts(ordered)

    def _drain_and_barrier(self, tick_clock, wait_clock):
        from bass_rust import ScopedClock

        nc = self.nc
        probe = nc.sync.nop(hint="tail_wait_probe")
        wait_clock.add_sem_waits(
            probe.ins, ScopedClock({None: tick_clock.global_clock})
        )
        si = probe.ins.sync_info
        waits = list(si.on_wait) if si is not None else []
        probe.ins.sync_info = mybir.SyncInfo(on_wait=waits[:1], on_update=[])
        for w in waits[1:]:
            n = nc.sync.nop(hint="tail_wait_split")
            n.ins.sync_info = mybir.SyncInfo(on_wait=[w], on_update=[])
        nc.sync.drain()
        nc.all_engine_barrier()
        assert self.sems is not None
        popped = nc._tile_sem_poison_stack.pop()
        assert popped is self._sem_poison
        nc.clear_and_free_semaphores(list(self.sems.allocated().values()))
        nc.all_engine_barrier()


def _build_bass(mode):
    """mode: how the natural-layout QK pass (which feeds attn) multiplies.
    "bf16": single bf16 product (attn err ~1e-4 measured)
    "hilo": bf16 hi/lo 3-term split (attn err ~1e-5, 3x the natural MMs)
    The transposed pass + AV matmul (which feed only `out`) are always
    single bf16."""
    nc = bass.Bass()
    EXP = mybir.ActivationFunctionType.Exp

    if mode == "hilo":
        qloT = nc.dram_tensor("qloT", [HPC, D, S], BF16, kind="ExternalInput")
        kloT = nc.dram_tensor("kloT", [HPC, D, S], BF16, kind="ExternalInput")
    qsTb = nc.dram_tensor("qsTb", [HPC, D, S], BF16, kind="ExternalInput")
    kTb = nc.dram_tensor("kTb", [HPC, D, S], BF16, kind="ExternalInput")
    vaug = nc.dram_tensor("vaug", [HPC, S, D + 1], BF16, kind="ExternalInput")
    attn = nc.dram_tensor("attn", [HPC, S, S], F32, kind="ExternalOutput")
    oden = nc.dram_tensor("oden", [HPC, D + 1, S], F32, kind="ExternalOutput")

    with _TileContextPatched(nc) as tc:
        with (
            tc.tile_pool(name="qk_in", bufs=3) as qk_pool,
            tc.tile_pool(name="v_in", bufs=3) as v_pool,
            tc.tile_pool(name="eT", bufs=4) as eT_pool,
            tc.tile_pool(name="enat", bufs=4) as en_pool,
            tc.tile_pool(name="attn_sb", bufs=4) as at_pool,
            tc.tile_pool(name="oden_sb", bufs=2) as od_pool,
            tc.tile_pool(name="small", bufs=8) as sm_pool,
            tc.tile_pool(name="ps_sc", bufs=3, space="PSUM") as ps_sc,
            tc.tile_pool(name="ps_od", bufs=1, space="PSUM") as ps_od,
        ):
            for h in range(HPC):
                qsb = qk_pool.tile([D, S], BF16, tag="qsb")
                ktb = qk_pool.tile([D, S], BF16, tag="ktb")
                nc.sync.dma_start(out=qsb[:], in_=qsTb[h])
                nc.sync.dma_start(out=ktb[:], in_=kTb[h])
                if mode == "hilo":
                    qlo = qk_pool.tile([D, S], BF16, tag="qlo")
                    klo = qk_pool.tile([D, S], BF16, tag="klo")
                    nc.sync.dma_start(out=qlo[:], in_=qloT[h])
                    nc.sync.dma_start(out=klo[:], in_=kloT[h])
                vt = v_pool.tile([P, NT, D + 1], BF16)
                nc.sync.dma_start(
                    out=vt[:], in_=vaug[h].rearrange("(t p) c -> p t c", p=P)
                )

                # --- interleaved transposed + natural passes (the natural
                # pass is independent of the transposed one, so alternating
                # per key/query tile keeps the ACT exp backlog at ~1 tile
                # instead of 8 at the end of each head) ---
                od_ps = ps_od.tile([D + 1, S], F32)
                for ktile in range(NT):
                    # transposed unit: logits^T -> e^T -> outAug^T accum
                    sT_ps = ps_sc.tile([P, S], F32, tag="sc")
                    for j in range(2):
                        nc.tensor.matmul(
                            sT_ps[:, j * MM_HALF : (j + 1) * MM_HALF],
                            lhsT=ktb[:, ktile * P : (ktile + 1) * P],
                            rhs=qsb[:, j * MM_HALF : (j + 1) * MM_HALF],
                            start=True,
                            stop=True,
                            skip_group_check=True,
                        )
                    eT = eT_pool.tile([P, S], BF16)
                    nc.scalar.activation(out=eT[:], in_=sT_ps[:], func=EXP)
                    for j in range(2):
                        nc.tensor.matmul(
                            od_ps[:, j * MM_HALF : (j + 1) * MM_HALF],
                            lhsT=vt[:, ktile, :],
                            rhs=eT[:, j * MM_HALF : (j + 1) * MM_HALF],
                            start=(ktile == 0),
                            stop=(ktile == NT - 1),
                            skip_group_check=True,
                        )
                    # natural unit (qt == ktile): exp+accum, normalize, DMA
                    qt = ktile
                    s_ps = ps_sc.tile([P, S], F32, tag="sc")
                    q_sl = slice(qt * P, (qt + 1) * P)
                    for j in range(2):
                        sl_out = s_ps[:, j * MM_HALF : (j + 1) * MM_HALF]
                        k_sl = slice(j * MM_HALF, (j + 1) * MM_HALF)
                        if mode == "hilo":
                            nc.tensor.matmul(
                                sl_out, lhsT=qsb[:, q_sl], rhs=ktb[:, k_sl],
                                start=True, stop=False, skip_group_check=True,
                            )
                            nc.tensor.matmul(
                                sl_out, lhsT=qlo[:, q_sl], rhs=ktb[:, k_sl],
                                start=False, stop=False, skip_group_check=True,
                            )
                            nc.tensor.matmul(
                                sl_out, lhsT=qsb[:, q_sl], rhs=klo[:, k_sl],
                                start=False, stop=True, skip_group_check=True,
                            )
                        else:
                            nc.tensor.matmul(
                                sl_out, lhsT=qsb[:, q_sl], rhs=ktb[:, k_sl],
                                start=True, stop=True, skip_group_check=True,
                            )
                    en = en_pool.tile([P, S], F32)
                    dn = sm_pool.tile([P, 1], F32, tag="dn")
                    nc.scalar.activation(
                        out=en[:], in_=s_ps[:], func=EXP, accum_out=dn[:]
                    )
                    invd = sm_pool.tile([P, 1], F32, tag="invd")
                    nc.vector.reciprocal(out=invd[:], in_=dn[:])
                    at = at_pool.tile([P, S], F32)
                    nc.vector.tensor_scalar_mul(at[:], en[:], invd[:])
                    nc.sync.dma_start(
                        out=attn[h, qt * P : (qt + 1) * P, :], in_=at[:]
                    )
                od_sb = od_pool.tile([D + 1, S], F32)
                nc.vector.tensor_copy(out=od_sb[:], in_=od_ps[:])
                nc.sync.dma_start(out=oden[h], in_=od_sb[:])
    return nc


_nc_cache = {}


def _get_nc(mode):
    if mode not in _nc_cache:
        _nc_cache[mode] = _build_bass(mode)
    return _nc_cache[mode]


def _ensure_ntff_hook():
    """bass_utils' trace path imports antenv.axon_hooks, which this
    container does not ship.  Recreate it from the boot helper so NTFF
    profiling (HW exec time) works.  Best-effort: failures just mean no
    trace."""
    try:
        from antenv.axon_hooks import get_axon_ntff_profile_hook  # noqa: F401

        return
    except ImportError:
        pass
    try:
        import types

        import antenv
        from trn_agent_boot.trn_boot import _ntff_profile_via_ctypes

        hook = {"h": _ntff_profile_via_ctypes("/opt/axon/libaxon_pjrt.so")}
        m = types.ModuleType("antenv.axon_hooks")
        m.get_axon_ntff_profile_hook = lambda: hook["h"]

        def _set(h):
            hook["h"] = h

        m.set_axon_ntff_profile_hook = _set
        sys.modules["antenv.axon_hooks"] = m
        antenv.axon_hooks = m
    except Exception:
        pass


def _host_reference_fallback(q, k, v):
    """Exact (host) computation, used only if the eps clamp could bind."""
    s = np.einsum("bhqd,bhkd->bhqk", q / 8.0, k).astype(np.float32)
    s = s / np.abs(s.min())
    rn = np.linalg.norm(s, axis=-1, keepdims=True)
    s = s / np.maximum(rn, 1e-12)
    s = s - s.max(axis=-1, keepdims=True)
    e = np.exp(s)
    attn = e / e.sum(axis=-1, keepdims=True)
    out = np.einsum("bhqk,bhkd->bhqd", attn, v).astype(np.float32)
    return out, attn.astype(np.float32)


def kernel(q, k, v):
    global last_results
    q = np.asarray(q, dtype=np.float32)
    k = np.asarray(k, dtype=np.float32)
    v = np.asarray(v, dtype=np.float32)

    qf = q.reshape(BH, S, D).astype(np.float64)
    kf = k.reshape(BH, S, D).astype(np.float64)
    # row norms of q @ k^T via the Gram matrix (exact to f64)
    G = np.matmul(kf.transpose(0, 2, 1), kf)  # [BH, D, D]
    w = np.matmul(qf, G)  # [BH, S, D]
    r2 = np.einsum("hsd,hsd->hs", qf, w)
    r = np.sqrt(np.maximum(r2, 0.0))
    # eps-clamp guard: the reference's F.normalize clamp binds for row q iff
    # ||s_q|| / (8*|m|) < 1e-12 with s = q@k^T and |m| = |min(s/8)|.  Since
    # |m| <= max_j |s_qj|/8 <= max_q ||s_q||/8, binding requires
    # r.min() < 1e-12 * r.max().  Check with a 10x safety margin.
    if r.min() < 1e-11 * r.max():
        out, attn = _host_reference_fallback(q, k, v)
        return out, attn, v

    import ml_dtypes

    bf16 = ml_dtypes.bfloat16
    mode = os.environ.get("BASS_QK_MODE", "bf16")
    u = 1.0 / r
    qsT = np.ascontiguousarray(
        (qf * u[:, :, None]).transpose(0, 2, 1), dtype=np.float32
    )  # [BH, D, S]
    kT = np.ascontiguousarray(
        k.reshape(BH, S, D).transpose(0, 2, 1)
    )  # [BH, D, S] f32
    vaug = np.concatenate(
        [v.reshape(BH, S, D), np.ones((BH, S, 1), dtype=np.float32)], axis=2
    ).astype(bf16)  # [BH, S, D+1] bf16 (feeds only the out matmul)
    qsTb = qsT.astype(bf16)
    kTb = kT.astype(bf16)

    nc = _get_nc(mode)
    shards = {
        "qsTb": qsTb,
        "kTb": kTb,
        "vaug": vaug,
    }
    if mode == "hilo":
        shards["qloT"] = (qsT - qsTb.astype(np.float32)).astype(bf16)
        shards["kloT"] = (kT - kTb.astype(np.float32)).astype(bf16)
    in_maps = [
        {
            name: np.ascontiguousarray(arr[c * HPC : (c + 1) * HPC])
            for name, arr in shards.items()
        }
        for c in range(NCORES)
    ]
    trace = bool(int(os.environ.get("BASS_KERNEL_TRACE", "0")))
    if trace:
        _ensure_ntff_hook()
    res = run_bass_kernel_spmd(
        nc, in_maps, core_ids=list(range(NCORES)), trace=trace
    )
    last_results = res

    attn = (
        np.concatenate([r_["attn"] for r_ in res.results], axis=0)
        .reshape(B, H, S, S)
    )
    oden = np.concatenate([r_["oden"] for r_ in res.results], axis=0)
    out = np.ascontiguousarray(
        (oden[:, :D, :] / oden[:, D : D + 1, :]).transpose(0, 2, 1)
    ).reshape(B, H, S, D)
    return out, attn, v
